# revision 27
# baseline (speedup 1.0000x reference)
"""Attention pooling kernel for Trainium2 (Bass/Tile), SPMD over 8 NeuronCores.

Reference computation (per batch b):
    scores[t] = x[b,t,:] @ q / sqrt(D) + (1-mask[b,t]) * (-1e9)
    attn      = softmax(scores)            # over t
    out[b,:]  = sum_t attn[t] * x[b,t,:]

Strategy: data-parallel over batch (4 batches per core). One pass over x
(read once from HBM, 64 MiB/core -> ~190us DMA floor at 358 GB/s):
  - x[b] viewed as [128 partitions, 64 cols, 512] with t = p*64 + n,
    streamed in [128, CHUNK, 512] fp32 chunks (4 MiB, 32 KB contiguous per
    partition).
  - scores: per tile [128, 512], s[p] = sum_d x[p,d]*q[d]. Split across
    engines to keep DVE (the pacing engine) under the DMA floor:
      * DVE tiles: fused tensor_tensor+accum (x*q bcast, reduce over d).
      * GpSimd tiles: multiply on GpSimd, reduce via ScalarE Copy+accum.
  - mask bias: negm = (m-1)*1e9 via one ScalarE activation per batch;
    one DVE add per chunk; exp on ScalarE with the 1/sqrt(D) scale fused
    (scores are O(0.1) so no max-subtraction; masked lanes exp -> 0).
  - pooled accumulation on PE: psum[1,512] += exp_col.T @ x_tile over all
    64 tiles of a batch. CHUNK=16 keeps PE matmul bursts long enough
    (>3us) to ramp the PE p-state to full clock. Z = sum(exp) via
    ones-matmul; out = acc * (1/Z) on ScalarE.
"""

import os

import numpy as np

import bass_rust as _br
import concourse.bass as bass
import concourse.tile as tile
from concourse import mybir
from concourse.bass_utils import run_bass_kernel_spmd

B, T, D = 32, 8192, 512
N_CORES = 8
BC = B // N_CORES  # batches per core
P = 128  # SBUF partitions
NCOL = T // P  # 64 tiles (columns) per batch
CHUNK = int(os.environ.get("AP_CHUNK", "8"))  # tiles per DMA chunk
NCHUNK = NCOL // CHUNK
# Tiles accumulate across DMA chunks and flush (bias+exp+PE matmuls) per
# SUPER tiles: DMA stays at the 2 MiB sweet spot (~356 GB/s measured; 4 MiB
# chunks drop to ~332) while PE matmul bursts stay long.
SUPER = int(os.environ.get("AP_SUPER", "32"))
BIG = 1.0e9
SCALE = 1.0 / float(np.sqrt(np.float32(D)))

F32 = mybir.dt.float32
I32 = mybir.dt.int32

# Matmul input dtype for the pooling accumulation (PE). float32r (TF32-style
# rounded fp32) runs at 1 cycle/row for N>=256 and is bit-identical to fp32
# in SBUF, so the plain HWDGE DMA path works with no cast.
MM_DTYPE = os.environ.get("AP_MM_DTYPE", "float32r")
# Score-path split per chunk. GpSimd is NOT used: its SBUF port is shared
# with VectorE, so concurrent GpSimd elementwise work slows DVE ~2x
# (measured: STT 686 -> 1241 ns). Instead, FUSED tiles use the DVE
# one-op multiply+reduce; the rest are multiplied on DVE in groups of
# GROUP tiles (one tensor_tensor, amortized overhead) and reduced on
# ScalarE (own SBUF port) via Copy+accum.
# Fused (one-op DVE) tiles per SUPER group; the rest are DVE-multiplied in
# small groups and reduced on ScalarE.
FUSED = int(os.environ.get("AP_FUSED", "16"))
XT_BUFS = int(os.environ.get("AP_XT_BUFS", "6"))
# Score path precision: with AP_BF16=1, ScalarE casts each x sub-chunk to
# bf16 (one cheap activation) and the fused DVE reduce runs in bf16, where
# the DVE 2x_1P perf mode halves the streaming time. The pooling matmul
# still consumes the fp32r tiles, so only the softmax weights see bf16.
BF16_SCORES = os.environ.get("AP_BF16", "1") == "1"
BF16 = mybir.dt.bfloat16

_TAPERS = {16: [2, 2, 4, 8], 8: [2, 2, 4], 4: [2, 2]}


def _chunk_plan(batch, n_batches):
    """Per-batch list of (start, size) DMA chunks. The first/last batch
    taper so the pipeline head (first scores wait on the first DMA) and
    tail (post-DMA compute of the final chunk) stay short."""
    taper = _TAPERS.get(CHUNK, [CHUNK])
    assert sum(taper) == CHUNK
    if n_batches > 1 and batch == 0:
        sizes = taper + [CHUNK] * (NCHUNK - 1)
    elif batch == n_batches - 1:
        sizes = [CHUNK] * (NCHUNK - 1) + taper[::-1]
    else:
        sizes = [CHUNK] * NCHUNK
    plan, pos = [], 0
    for sz in sizes:
        plan.append((pos, sz))
        pos += sz
    return plan


def _split_groups(r):
    """Split r tiles into multiply-groups of width 2-3 (width 1 wastes the
    per-op overhead)."""
    out = []
    while r >= 2:
        take = 3 if r == 3 else 2
        out.append(take)
        r -= take
    if r:
        if out:
            out[-1] += 1
        else:
            out.append(1)
    return out
XT_DT = {
    "float32": mybir.dt.float32,
    "float32r": mybir.dt.float32r,
    "bfloat16": mybir.dt.bfloat16,
}[MM_DTYPE]


def _split_multi_waits(nc):
    """The walrus build in this container accepts only one sync-wait command
    per instruction; hoist extra waits onto standalone EventSemaphore
    instructions placed just before (same engine, program order preserved)."""
    for f in nc.m.functions:
        for b in f.blocks:
            insts = b.instructions
            new = []
            changed = False
            for inst in insts:
                si = inst.sync_info
                if si is not None and len(si.on_wait) > 1:
                    waits = list(si.on_wait)
                    for w in waits[:-1]:
                        ies = mybir.InstEventSemaphore(
                            name=f"I-waitsplit-{nc.next_id()}", ins=[], outs=[]
                        )
                        ies.engine = inst.engine
                        ies.sync_info = _br.SyncInfo(on_wait=[w], on_update=[])
                        new.append(ies)
                    inst.sync_info = _br.SyncInfo(
                        on_wait=[waits[-1]], on_update=list(si.on_update)
                    )
                    changed = True
                new.append(inst)
            if changed:
                b.instructions = new


def _build_bass():
    nc = bass.Bass(
        "TRN2", target_bir_lowering=False, debug=False, num_devices=N_CORES
    )
    x_dram_dt = mybir.dt.float32r if MM_DTYPE == "float32r" else F32
    x = nc.dram_tensor("x", [BC, T, D], x_dram_dt, kind="ExternalInput").ap()
    mask = nc.dram_tensor("mask", [BC, T], I32, kind="ExternalInput").ap()
    q = nc.dram_tensor("pool_query", [1, 1, D], F32, kind="ExternalInput").ap()
    out = nc.dram_tensor("out", [BC, D], F32, kind="ExternalOutput").ap()

    # t = p * NCOL + n  (partition-major): per-partition rows are contiguous
    # in DRAM, so a [128, CHUNK, 512] chunk is CHUNK*2 KB contiguous per
    # partition.
    xv = x.rearrange("b (p n) d -> b p n d", p=P)
    mv = mask.rearrange("b (p n) -> b p n", p=P)

    with tile.TileContext(nc) as tc:
        with (
            tc.tile_pool(name="const", bufs=1) as const_pool,
            tc.tile_pool(name="xp", bufs=XT_BUFS) as xpool,
            tc.tile_pool(name="dp", bufs=2) as dpool,
            tc.tile_pool(name="gp", bufs=3) as gpool,
            tc.tile_pool(name="bp", bufs=2) as bpool,
            tc.tile_pool(name="ep", bufs=2) as epool,
            tc.tile_pool(name="pacc", bufs=2, space="PSUM") as pacc,
            tc.tile_pool(name="pz", bufs=2, space="PSUM") as pz,
        ):
            # q broadcast to all 128 partitions (one-time, 256 KB)
            q_bcast = const_pool.tile([P, D], F32)
            q_src = bass.AP(tensor=q.tensor, offset=q.offset, ap=[[0, P], [1, D]])
            nc.gpsimd.dma_start(out=q_bcast, in_=q_src)

            ones_col = const_pool.tile([P, 1], F32)
            nc.gpsimd.memset(ones_col, 1.0)

            neg_big = const_pool.tile([P, 1], F32)
            nc.gpsimd.memset(neg_big, -BIG)

            q_bf16 = None
            if BF16_SCORES:
                q_bf16 = const_pool.tile([P, D], BF16)
                nc.scalar.copy(out=q_bf16, in_=q_bcast)

            # All mask loads up-front on the SWDGE (gpsimd) ring: the sync
            # HWDGE ring is an in-order FIFO, and anything interleaved with
            # the x stream would stall it.
            masks = []
            for b in range(BC):
                m_i32 = bpool.tile([P, NCOL], I32, tag=f"m{b}")
                nc.gpsimd.dma_start(out=m_i32, in_=mv[b])
                masks.append(m_i32)

            for b in range(BC):
                # mask -> additive bias: (m - 1) * 1e9  (0 valid, -1e9 pad),
                # one ScalarE op (casts i32 input, scale+bias fused).
                negm = bpool.tile([P, NCOL], F32)
                nc.scalar.activation(
                    out=negm,
                    in_=masks[b],
                    func=mybir.ActivationFunctionType.Identity,
                    scale=BIG,
                    bias=neg_big,
                )

                s_all = bpool.tile([P, NCOL], F32)
                exp_all = bpool.tile([P, NCOL], XT_DT)
                acc = pacc.tile([1, D], F32)
                z = pz.tile([1, 1], F32)

                # Tiles accumulate in `pending` as their scores are issued
                # and flush (mask bias + exp + PE matmul burst) per SUPER
                # tiles, decoupling DMA chunk size from PE burst length.
                pending = []

                def flush():
                    if not pending:
                        return
                    n0 = pending[0][0]
                    cs = slice(n0, n0 + len(pending))
                    nc.vector.tensor_tensor(
                        out=s_all[:, cs],
                        in0=s_all[:, cs],
                        in1=negm[:, cs],
                        op=mybir.AluOpType.add,
                    )
                    nc.scalar.activation(
                        out=exp_all[:, cs],
                        in_=s_all[:, cs],
                        func=mybir.ActivationFunctionType.Exp,
                    )
                    for n, mxt, j in pending:
                        nc.tensor.matmul(
                            acc,
                            lhsT=exp_all[:, n : n + 1],
                            rhs=mxt[:, j, :],
                            start=(n == 0),
                            stop=(n == NCOL - 1),
                        )
                    pending.clear()

                fc = 0  # full-chunk counter (for the fused-count pattern)
                m = max(1, SUPER // CHUNK)
                for start, size in _chunk_plan(b, BC):
                    xt = xpool.tile([P, size, D], XT_DT, tag="xt")
                    xdma = nc.sync if XT_DT == x_dram_dt else nc.gpsimd
                    xdma.dma_start(
                        out=xt, in_=xv[b, :, start : start + size, :]
                    )
                    if BF16_SCORES:
                        # one ScalarE cast for the whole sub-chunk
                        xb = xpool.tile([P, size, D], BF16, tag="xb")
                        nc.scalar.copy(out=xb, in_=xt)
                        s_in, s_q = xb, q_bf16
                        s_dt = BF16
                    else:
                        s_in, s_q = xt, q_bcast
                        s_dt = F32
                    # Small (head/tail) chunks run all-fused: the pure-DVE
                    # path has the shortest dependency chain. Full chunks
                    # spread FUSED tiles per SUPER across their chunks.
                    if size < CHUNK:
                        fused = size
                    else:
                        fused = FUSED * (fc + 1) // m - FUSED * fc // m
                        fc += 1
                    # FUSED tiles: one DVE op each (multiply + reduce)
                    for j in range(fused):
                        n = start + j
                        prod = dpool.tile([P, D], s_dt, tag="dprod")
                        # s_all[:, n] = sum_d x[:, n, d]*SCALE*q[d]
                        nc.vector.scalar_tensor_tensor(
                            out=prod,
                            in0=s_in[:, j, :],
                            scalar=SCALE,
                            in1=s_q,
                            op0=mybir.AluOpType.mult,
                            op1=mybir.AluOpType.mult,
                            accum_out=s_all[:, n : n + 1],
                        )
                    # Remaining tiles: DVE multiplies a group per op
                    # (q broadcast over the group via a zero-stride AP),
                    # ScalarE reduces each tile with the scale folded in.
                    g = fused
                    for gw in _split_groups(size - fused):
                        prod2 = gpool.tile([P, 3, D], F32, tag="gprod")
                        q_rep = bass.AP(
                            tensor=q_bcast.tensor,
                            offset=q_bcast.offset,
                            ap=[list(q_bcast.ap[0]), [0, gw], [1, D]],
                        )
                        nc.vector.tensor_tensor(
                            out=prod2[:, :gw, :],
                            in0=xt[:, g : g + gw, :],
                            in1=q_rep,
                            op=mybir.AluOpType.mult,
                        )
                        for i in range(gw):
                            n = start + g + i
                            nc.scalar.activation(
                                out=prod2[:, i, :],
                                in_=prod2[:, i, :],
                                func=mybir.ActivationFunctionType.Copy,
                                scale=SCALE,
                                accum_out=s_all[:, n : n + 1],
                            )
                        g += gw
                    for j in range(size):
                        pending.append((start + j, xt, j))
                    # Flush per SUPER; in the last batch's tail taper flush
                    # every sub-chunk so the post-DMA tail stays short.
                    tail_taper = b == BC - 1 and start + size > CHUNK * (NCHUNK - 1)
                    if len(pending) >= SUPER or tail_taper:
                        flush()
                flush()

                # Z = sum over all t of exp
                colsum = bpool.tile([P, 1], F32)
                ecopy = bpool.tile([P, NCOL], XT_DT, tag="ecopy")
                nc.scalar.activation(
                    out=ecopy,
                    in_=exp_all,
                    func=mybir.ActivationFunctionType.Copy,
                    accum_out=colsum,
                )
                nc.tensor.matmul(z, lhsT=colsum, rhs=ones_col, start=True, stop=True)

                zrec = epool.tile([1, 1], F32)
                nc.vector.reciprocal(zrec, z)
                out_row = epool.tile([1, D], F32)
                nc.scalar.mul(out=out_row, in_=acc, mul=zrec)
                # out goes via SWDGE so it never stalls the x-stream FIFO
                nc.gpsimd.dma_start(out=out[b : b + 1, :], in_=out_row)

    _split_multi_waits(nc)
    return nc


def _run(x, mask, pool_query, trace=False):
    x = np.ascontiguousarray(np.asarray(x, dtype=np.float32))
    mask = np.ascontiguousarray(np.asarray(mask, dtype=np.int32))
    pool_query = np.ascontiguousarray(np.asarray(pool_query, dtype=np.float32))
    assert x.shape == (B, T, D) and mask.shape == (B, T)

    nc = _build_bass()
    in_maps = []
    for c in range(N_CORES):
        lo, hi = c * BC, (c + 1) * BC
        in_maps.append(
            {
                "x": np.ascontiguousarray(x[lo:hi]),
                "mask": np.ascontiguousarray(mask[lo:hi]),
                "pool_query": pool_query,
            }
        )
    res = run_bass_kernel_spmd(
        nc, in_maps, core_ids=list(range(N_CORES)), trace=trace
    )
    out = np.concatenate([r["out"] for r in res.results], axis=0)
    return out, res


def kernel(x, mask, pool_query):
    out, _ = _run(x, mask, pool_query)
    return out


# revision 28
# speedup vs baseline: 1.5709x; 1.5709x over previous
"""Attention pooling kernel for Trainium2 (Bass/Tile), SPMD over 8 NeuronCores.

Reference computation (per batch b):
    scores[t] = x[b,t,:] @ q / sqrt(D) + (1-mask[b,t]) * (-1e9)
    attn      = softmax(scores)            # over t
    out[b,:]  = sum_t attn[t] * x[b,t,:]

Strategy: data-parallel over batch (4 batches per core). One pass over x
(read once from HBM, 64 MiB/core -> ~190us DMA floor at 358 GB/s):
  - x[b] viewed as [128 partitions, 64 cols, 512] with t = p*64 + n,
    streamed in [128, CHUNK, 512] fp32 chunks (4 MiB, 32 KB contiguous per
    partition).
  - scores: per tile [128, 512], s[p] = sum_d x[p,d]*q[d]. Split across
    engines to keep DVE (the pacing engine) under the DMA floor:
      * DVE tiles: fused tensor_tensor+accum (x*q bcast, reduce over d).
      * GpSimd tiles: multiply on GpSimd, reduce via ScalarE Copy+accum.
  - mask bias: negm = (m-1)*1e9 via one ScalarE activation per batch;
    one DVE add per chunk; exp on ScalarE with the 1/sqrt(D) scale fused
    (scores are O(0.1) so no max-subtraction; masked lanes exp -> 0).
  - pooled accumulation on PE: psum[1,512] += exp_col.T @ x_tile over all
    64 tiles of a batch. CHUNK=16 keeps PE matmul bursts long enough
    (>3us) to ramp the PE p-state to full clock. Z = sum(exp) via
    ones-matmul; out = acc * (1/Z) on ScalarE.
"""

import os

import numpy as np

import bass_rust as _br
import concourse.bass as bass
import concourse.tile as tile
from concourse import mybir
from concourse.bass_utils import run_bass_kernel_spmd

B, T, D = 32, 8192, 512
N_CORES = 8
BC = B // N_CORES  # batches per core
P = 128  # SBUF partitions
NCOL = T // P  # 64 tiles (columns) per batch
CHUNK = int(os.environ.get("AP_CHUNK", "8"))  # tiles per DMA chunk
NCHUNK = NCOL // CHUNK
# Tiles accumulate across DMA chunks and flush (bias+exp+PE matmuls) per
# SUPER tiles: DMA stays at the 2 MiB sweet spot (~356 GB/s measured; 4 MiB
# chunks drop to ~332) while PE matmul bursts stay long.
SUPER = int(os.environ.get("AP_SUPER", "16"))
BIG = 1.0e9
SCALE = 1.0 / float(np.sqrt(np.float32(D)))

F32 = mybir.dt.float32
I32 = mybir.dt.int32

# Matmul input dtype for the pooling accumulation (PE). float32r (TF32-style
# rounded fp32) runs at 1 cycle/row for N>=256 and is bit-identical to fp32
# in SBUF, so the plain HWDGE DMA path works with no cast.
MM_DTYPE = os.environ.get("AP_MM_DTYPE", "float32r")
# Score-path split per chunk. GpSimd is NOT used: its SBUF port is shared
# with VectorE, so concurrent GpSimd elementwise work slows DVE ~2x
# (measured: STT 686 -> 1241 ns). Instead, FUSED tiles use the DVE
# one-op multiply+reduce; the rest are multiplied on DVE in groups of
# GROUP tiles (one tensor_tensor, amortized overhead) and reduced on
# ScalarE (own SBUF port) via Copy+accum.
# Fused (one-op DVE) tiles per SUPER group; the rest are DVE-multiplied in
# small groups and reduced on ScalarE.
FUSED = int(os.environ.get("AP_FUSED", "16"))
XT_BUFS = int(os.environ.get("AP_XT_BUFS", "6"))
# Score path precision: with AP_BF16=1, ScalarE casts each x sub-chunk to
# bf16 (one cheap activation) and the fused DVE reduce runs in bf16, where
# the DVE 2x_1P perf mode halves the streaming time. The pooling matmul
# still consumes the fp32r tiles, so only the softmax weights see bf16.
BF16_SCORES = os.environ.get("AP_BF16", "1") == "1"
BF16 = mybir.dt.bfloat16

_TAPERS = {16: [2, 2, 4, 8], 8: [2, 2, 4], 4: [2, 2]}


def _chunk_plan(batch, n_batches):
    """Per-batch list of (start, size) DMA chunks. The first/last batch
    taper so the pipeline head (first scores wait on the first DMA) and
    tail (post-DMA compute of the final chunk) stay short."""
    taper = _TAPERS.get(CHUNK, [CHUNK])
    assert sum(taper) == CHUNK
    if n_batches > 1 and batch == 0:
        sizes = taper + [CHUNK] * (NCHUNK - 1)
    elif batch == n_batches - 1:
        sizes = [CHUNK] * (NCHUNK - 1) + taper[::-1]
    else:
        sizes = [CHUNK] * NCHUNK
    plan, pos = [], 0
    for sz in sizes:
        plan.append((pos, sz))
        pos += sz
    return plan


def _split_groups(r):
    """Split r tiles into multiply-groups of width 2-3 (width 1 wastes the
    per-op overhead)."""
    out = []
    while r >= 2:
        take = 3 if r == 3 else 2
        out.append(take)
        r -= take
    if r:
        if out:
            out[-1] += 1
        else:
            out.append(1)
    return out
XT_DT = {
    "float32": mybir.dt.float32,
    "float32r": mybir.dt.float32r,
    "bfloat16": mybir.dt.bfloat16,
}[MM_DTYPE]


def _split_multi_waits(nc):
    """The walrus build in this container accepts only one sync-wait command
    per instruction; hoist extra waits onto standalone EventSemaphore
    instructions placed just before (same engine, program order preserved)."""
    for f in nc.m.functions:
        for b in f.blocks:
            insts = b.instructions
            new = []
            changed = False
            for inst in insts:
                si = inst.sync_info
                if si is not None and len(si.on_wait) > 1:
                    waits = list(si.on_wait)
                    for w in waits[:-1]:
                        ies = mybir.InstEventSemaphore(
                            name=f"I-waitsplit-{nc.next_id()}", ins=[], outs=[]
                        )
                        ies.engine = inst.engine
                        ies.sync_info = _br.SyncInfo(on_wait=[w], on_update=[])
                        new.append(ies)
                    inst.sync_info = _br.SyncInfo(
                        on_wait=[waits[-1]], on_update=list(si.on_update)
                    )
                    changed = True
                new.append(inst)
            if changed:
                b.instructions = new


def _build_bass():
    nc = bass.Bass(
        "TRN2", target_bir_lowering=False, debug=False, num_devices=N_CORES
    )
    x_dram_dt = mybir.dt.float32r if MM_DTYPE == "float32r" else F32
    x = nc.dram_tensor("x", [BC, T, D], x_dram_dt, kind="ExternalInput").ap()
    mask = nc.dram_tensor("mask", [BC, T], I32, kind="ExternalInput").ap()
    q = nc.dram_tensor("pool_query", [1, 1, D], F32, kind="ExternalInput").ap()
    out = nc.dram_tensor("out", [BC, D], F32, kind="ExternalOutput").ap()

    # t = p * NCOL + n  (partition-major): per-partition rows are contiguous
    # in DRAM, so a [128, CHUNK, 512] chunk is CHUNK*2 KB contiguous per
    # partition.
    xv = x.rearrange("b (p n) d -> b p n d", p=P)
    mv = mask.rearrange("b (p n) -> b p n", p=P)

    with tile.TileContext(nc) as tc:
        with (
            tc.tile_pool(name="const", bufs=1) as const_pool,
            tc.tile_pool(name="xp", bufs=XT_BUFS) as xpool,
            tc.tile_pool(name="dp", bufs=2) as dpool,
            tc.tile_pool(name="gp", bufs=3) as gpool,
            tc.tile_pool(name="bp", bufs=2) as bpool,
            tc.tile_pool(name="ep", bufs=2) as epool,
            tc.tile_pool(name="pacc", bufs=2, space="PSUM") as pacc,
            tc.tile_pool(name="pz", bufs=2, space="PSUM") as pz,
        ):
            # q broadcast to all 128 partitions (one-time, 256 KB)
            q_bcast = const_pool.tile([P, D], F32)
            q_src = bass.AP(tensor=q.tensor, offset=q.offset, ap=[[0, P], [1, D]])
            nc.gpsimd.dma_start(out=q_bcast, in_=q_src)

            ones_col = const_pool.tile([P, 1], F32)
            nc.vector.memset(ones_col, 1.0)

            neg_big = const_pool.tile([P, 1], F32)
            nc.vector.memset(neg_big, -BIG)

            q_bf16 = None
            if BF16_SCORES:
                q_bf16 = const_pool.tile([P, D], BF16)
                nc.scalar.copy(out=q_bf16, in_=q_bcast)

            # All mask loads up-front on the SWDGE (gpsimd) ring: the sync
            # HWDGE ring is an in-order FIFO, and anything interleaved with
            # the x stream would stall it.
            masks = []
            for b in range(BC):
                m_i32 = bpool.tile([P, NCOL], I32, tag=f"m{b}")
                nc.gpsimd.dma_start(out=m_i32, in_=mv[b])
                masks.append(m_i32)

            for b in range(BC):
                # mask -> additive bias: (m - 1) * 1e9  (0 valid, -1e9 pad),
                # one ScalarE op (casts i32 input, scale+bias fused).
                negm = bpool.tile([P, NCOL], F32)
                nc.scalar.activation(
                    out=negm,
                    in_=masks[b],
                    func=mybir.ActivationFunctionType.Identity,
                    scale=BIG,
                    bias=neg_big,
                )

                s_all = bpool.tile([P, NCOL], F32)
                exp_all = bpool.tile([P, NCOL], XT_DT)
                acc = pacc.tile([1, D], F32)
                z = pz.tile([1, 1], F32)

                # Tiles accumulate in `pending` as their scores are issued
                # and flush (mask bias + exp + PE matmul burst) per SUPER
                # tiles, decoupling DMA chunk size from PE burst length.
                pending = []

                def flush():
                    if not pending:
                        return
                    n0 = pending[0][0]
                    cs = slice(n0, n0 + len(pending))
                    nc.vector.tensor_tensor(
                        out=s_all[:, cs],
                        in0=s_all[:, cs],
                        in1=negm[:, cs],
                        op=mybir.AluOpType.add,
                    )
                    nc.scalar.activation(
                        out=exp_all[:, cs],
                        in_=s_all[:, cs],
                        func=mybir.ActivationFunctionType.Exp,
                    )
                    for n, mxt, j in pending:
                        nc.tensor.matmul(
                            acc,
                            lhsT=exp_all[:, n : n + 1],
                            rhs=mxt[:, j, :],
                            start=(n == 0),
                            stop=(n == NCOL - 1),
                        )
                    pending.clear()

                fc = 0  # full-chunk counter (for the fused-count pattern)
                m = max(1, SUPER // CHUNK)
                for start, size in _chunk_plan(b, BC):
                    xt = xpool.tile([P, size, D], XT_DT, tag="xt")
                    xdma = nc.sync if XT_DT == x_dram_dt else nc.gpsimd
                    xdma.dma_start(
                        out=xt, in_=xv[b, :, start : start + size, :]
                    )
                    if BF16_SCORES:
                        # one ScalarE cast for the whole sub-chunk
                        xb = xpool.tile([P, size, D], BF16, tag="xb")
                        nc.scalar.copy(out=xb, in_=xt)
                        s_in, s_q = xb, q_bf16
                        s_dt = BF16
                    else:
                        s_in, s_q = xt, q_bcast
                        s_dt = F32
                    # Small (head/tail) chunks run all-fused: the pure-DVE
                    # path has the shortest dependency chain. Full chunks
                    # spread FUSED tiles per SUPER across their chunks.
                    if size < CHUNK:
                        fused = size
                    else:
                        fused = FUSED * (fc + 1) // m - FUSED * fc // m
                        fc += 1
                    # FUSED tiles: one DVE op each (multiply + reduce)
                    for j in range(fused):
                        n = start + j
                        prod = dpool.tile([P, D], s_dt, tag="dprod")
                        # s_all[:, n] = sum_d x[:, n, d]*SCALE*q[d]
                        nc.vector.scalar_tensor_tensor(
                            out=prod,
                            in0=s_in[:, j, :],
                            scalar=SCALE,
                            in1=s_q,
                            op0=mybir.AluOpType.mult,
                            op1=mybir.AluOpType.mult,
                            accum_out=s_all[:, n : n + 1],
                        )
                    # Remaining tiles: DVE multiplies a group per op
                    # (q broadcast over the group via a zero-stride AP),
                    # ScalarE reduces each tile with the scale folded in.
                    g = fused
                    for gw in _split_groups(size - fused):
                        prod2 = gpool.tile([P, 3, D], F32, tag="gprod")
                        q_rep = bass.AP(
                            tensor=q_bcast.tensor,
                            offset=q_bcast.offset,
                            ap=[list(q_bcast.ap[0]), [0, gw], [1, D]],
                        )
                        nc.vector.tensor_tensor(
                            out=prod2[:, :gw, :],
                            in0=xt[:, g : g + gw, :],
                            in1=q_rep,
                            op=mybir.AluOpType.mult,
                        )
                        for i in range(gw):
                            n = start + g + i
                            nc.scalar.activation(
                                out=prod2[:, i, :],
                                in_=prod2[:, i, :],
                                func=mybir.ActivationFunctionType.Copy,
                                scale=SCALE,
                                accum_out=s_all[:, n : n + 1],
                            )
                        g += gw
                    for j in range(size):
                        pending.append((start + j, xt, j))
                    # Flush per SUPER; in the last batch's tail taper flush
                    # every sub-chunk so the post-DMA tail stays short.
                    tail_taper = b == BC - 1 and start + size > CHUNK * (NCHUNK - 1)
                    if len(pending) >= SUPER or tail_taper:
                        flush()
                flush()

                # Z = sum over all t of exp
                colsum = bpool.tile([P, 1], F32)
                nc.vector.reduce_sum(colsum, exp_all, axis=mybir.AxisListType.X)
                nc.tensor.matmul(z, lhsT=colsum, rhs=ones_col, start=True, stop=True)

                zrec = epool.tile([1, 1], F32)
                nc.vector.reciprocal(zrec, z)
                out_row = epool.tile([1, D], F32)
                nc.scalar.mul(out=out_row, in_=acc, mul=zrec)
                # out goes via SWDGE so it never stalls the x-stream FIFO
                nc.gpsimd.dma_start(out=out[b : b + 1, :], in_=out_row)

    _split_multi_waits(nc)
    return nc


def _run(x, mask, pool_query, trace=False):
    x = np.ascontiguousarray(np.asarray(x, dtype=np.float32))
    mask = np.ascontiguousarray(np.asarray(mask, dtype=np.int32))
    pool_query = np.ascontiguousarray(np.asarray(pool_query, dtype=np.float32))
    assert x.shape == (B, T, D) and mask.shape == (B, T)

    nc = _build_bass()
    in_maps = []
    for c in range(N_CORES):
        lo, hi = c * BC, (c + 1) * BC
        in_maps.append(
            {
                "x": np.ascontiguousarray(x[lo:hi]),
                "mask": np.ascontiguousarray(mask[lo:hi]),
                "pool_query": pool_query,
            }
        )
    res = run_bass_kernel_spmd(
        nc, in_maps, core_ids=list(range(N_CORES)), trace=trace
    )
    out = np.concatenate([r["out"] for r in res.results], axis=0)
    return out, res


def kernel(x, mask, pool_query):
    out, _ = _run(x, mask, pool_query)
    return out
